# revision 17
# baseline (speedup 1.0000x reference)
"""Grouped MoE MLP (64 experts) on 8 Trainium2 NeuronCores.

Strategy: expert parallelism. Each core owns 8 experts (size-sorted "snake"
assignment so every core gets the same per-slot padded token capacity and the
padding is tight). Host pre-swizzles all tensors so every DMA is contiguous
per partition:

    w1s [S, 128(hi), KO, F]  bf16  - one 4.2 MB DMA per slot (32 KB/partition)
    w2s [S, 128(fi), FO, H]  bf16  - one 4.2 MB DMA per slot
    xT  [128(hi), KO, CTOT]  bf16  - one DMA, resident in SBUF
    outT[128(oi), OO, CTOT]  bf16  - one batched DMA per slot token-block

Compute per slot (weights stationary, tokens moving):
    hT[f, t]   = w1t[e] (stationary [h,f] tiles) @ xT (moving [h, t])
    hT         = gelu(hT)                    (ScalarE, PSUM f32 -> SBUF bf16)
    outT[o, t] = w2[e] (stationary [f,o] tiles) @ hT (moving [f, t])

PSUM accumulates in f32, output is written bf16 transposed and un-permuted /
upcast on host.
"""

import numpy as np

NCORES = 8
SLOTS = 8  # experts per core
NE = 64
H = 1024
F = 2048
T = 16384
P = 128
KO = H // P  # 8  k-tiles for mm1 (contraction over H)
FO = F // P  # 16 f-tiles (mm1 output tiles / mm2 contraction)
OO = H // P  # 8  output h-tiles for mm2
NMAX = 512  # max moving-operand length (one fp32 PSUM bank)

ACT_FN = "Gelu"  # overridable for CoreSim tests (Gelu not implemented there)

_prog_cache = {}


def _build_program(C, repeat=1):
    """Build the SPMD Bass program for per-slot token capacities C (len SLOTS).

    repeat>1 duplicates the compute body (same I/O) for slope timing in test.py.
    """
    from contextlib import ExitStack

    import concourse.tile as tile
    from concourse import bacc, mybir
    from concourse.bass import MemorySpace

    bf16 = mybir.dt.bfloat16
    f32 = mybir.dt.float32
    CTOT = int(sum(C))
    Cmax = int(max(C))
    CmaxB = min(Cmax, NMAX)  # token-block tile width

    nc = bacc.Bacc("TRN2", target_bir_lowering=False, debug=False, num_devices=NCORES)
    w1_d = nc.dram_tensor("w1s", [SLOTS, P, KO, F], bf16, kind="ExternalInput").ap()
    w2_d = nc.dram_tensor("w2s", [SLOTS, P, FO, H], bf16, kind="ExternalInput").ap()
    xT_d = nc.dram_tensor("xT", [SLOTS, P, KO, Cmax], bf16, kind="ExternalInput").ap()
    outT_d = nc.dram_tensor("outT", [P, OO, CTOT], bf16, kind="ExternalOutput").ap()

    with tile.TileContext(nc) as tc, ExitStack() as ctx:
        w1_pool = ctx.enter_context(tc.tile_pool(name="w1", bufs=2))
        w2_pool = ctx.enter_context(tc.tile_pool(name="w2", bufs=2))
        x_pool = ctx.enter_context(tc.tile_pool(name="x", bufs=3))
        h_pool = ctx.enter_context(tc.tile_pool(name="h", bufs=2))
        o_pool = ctx.enter_context(tc.tile_pool(name="o", bufs=2))
        ph_pool = ctx.enter_context(
            tc.tile_pool(name="ph", bufs=4, space=MemorySpace.PSUM)
        )
        po_pool = ctx.enter_context(
            tc.tile_pool(name="po", bufs=4, space=MemorySpace.PSUM)
        )

        # Startup: gate the first matmuls on ~1.7 MB of DMA instead of the
        # full 8.5 MB — slot-0's x first, then slot-0's w1 in f-chunks so
        # mm1 can start after the first chunk. Everything else streams
        # just-in-time in per-slot issue order.
        x_first = x_pool.tile([P, KO, Cmax], bf16, tag="x")
        nc.sync.dma_start(x_first, xT_d[0])
        w1_first = w1_pool.tile([P, KO, F], bf16, tag="w1")
        for fc in range(0, F, 512):
            nc.sync.dma_start(w1_first[:, :, fc : fc + 512], w1_d[0][:, :, fc : fc + 512])

        for _rep in range(repeat):
            off = 0
            for j in range(SLOTS):
                Cj = int(C[j])
                if _rep == 0 and j == 0:
                    x_sb = x_first
                    w1_sb = w1_first
                else:
                    x_sb = x_pool.tile([P, KO, Cmax], bf16, tag="x")
                    nc.sync.dma_start(x_sb, xT_d[j])
                    w1_sb = w1_pool.tile([P, KO, F], bf16, tag="w1")
                    nc.sync.dma_start(w1_sb, w1_d[j])
                w2_sb = w2_pool.tile([P, FO, H], bf16, tag="w2")
                nc.sync.dma_start(w2_sb, w2_d[j])

                for nb in range(0, Cj, NMAX):
                    NB = min(NMAX, Cj - nb)
                    h_sb = h_pool.tile([P, FO, CmaxB], bf16, tag="h")
                    for fo in range(FO):
                        ph = ph_pool.tile([P, NMAX], f32, tag="ph")
                        for k in range(KO):
                            nc.tensor.matmul(
                                ph[:, :NB],
                                w1_sb[:, k, fo * P : (fo + 1) * P],
                                x_sb[:, k, nb : nb + NB],
                                start=(k == 0),
                                stop=(k == KO - 1),
                            )
                        nc.scalar.activation(
                            h_sb[:, fo, :NB],
                            ph[:, :NB],
                            getattr(mybir.ActivationFunctionType, ACT_FN),
                        )
                    o_sb = o_pool.tile([P, OO, CmaxB], bf16, tag="o")
                    for oo in range(OO):
                        po = po_pool.tile([P, NMAX], f32, tag="po")
                        for fo in range(FO):
                            nc.tensor.matmul(
                                po[:, :NB],
                                w2_sb[:, fo, oo * P : (oo + 1) * P],
                                h_sb[:, fo, :NB],
                                start=(fo == 0),
                                stop=(fo == FO - 1),
                            )
                        nc.vector.tensor_copy(o_sb[:, oo, :NB], po[:, :NB])
                    if j == SLOTS - 1:
                        # split the last slot's output DMA so the kernel tail
                        # drains as the second half of copies completes
                        nc.sync.dma_start(
                            outT_d[:, : OO // 2, off + nb : off + nb + NB],
                            o_sb[:, : OO // 2, :NB],
                        )
                        nc.sync.dma_start(
                            outT_d[:, OO // 2 :, off + nb : off + nb + NB],
                            o_sb[:, OO // 2 :, :NB],
                        )
                    else:
                        nc.sync.dma_start(
                            outT_d[:, :, off + nb : off + nb + NB], o_sb[:, :, :NB]
                        )
                off += Cj

    nc.compile()
    return nc


def _get_program(C, repeat=1):
    key = (tuple(int(c) for c in C), repeat)
    if key not in _prog_cache:
        _prog_cache[key] = _build_program(key[0], repeat=repeat)
    return _prog_cache[key]


def plan(sizes):
    """Expert->core/slot assignment + slot capacities from token counts."""
    sizes = np.asarray(sizes, np.int64)
    assert sizes.shape == (NE,) and sizes.sum() == T
    order = np.argsort(-sizes, kind="stable")  # descending
    # expert_of[core][slot]
    expert_of = [[int(order[s * NCORES + c]) for s in range(SLOTS)] for c in range(NCORES)]
    C = []
    for s in range(SLOTS):
        m = max(int(sizes[order[s * NCORES + c]]) for c in range(NCORES))
        C.append(max(16, -(-m // 4) * 4))  # round up to multiple of 4, min 16
    offs = np.concatenate([[0], np.cumsum(C)]).astype(np.int64)
    return expert_of, C, offs


def prepare_inputs(x, w1, w2, sizes, expert_of, C, offs):
    """Host-side shard/pad/swizzle/cast. Returns per-core input maps."""
    import ml_dtypes

    bf16 = ml_dtypes.bfloat16
    x = np.asarray(x, np.float32)
    tok_offs = np.concatenate([[0], np.cumsum(sizes)]).astype(np.int64)
    w1_bf = np.asarray(w1, np.float32).astype(bf16)  # [NE, F, H]
    w2_bf = np.asarray(w2, np.float32).astype(bf16)  # [NE, F, H]
    CTOT = int(sum(C))

    in_maps = []
    for c in range(NCORES):
        experts = expert_of[c]
        # w1s[s, hi, ko, f] = w1[e].T[ko*128+hi, f]
        w1t_c = w1_bf[experts].transpose(0, 2, 1)  # [S, H, F]
        w1s = np.ascontiguousarray(
            w1t_c.reshape(SLOTS, KO, P, F).transpose(0, 2, 1, 3)
        )  # [S, 128, KO, F]
        # w2s[s, fi, fo, h] = w2[e][fo*128+fi, h]
        w2s = np.ascontiguousarray(
            w2_bf[experts].reshape(SLOTS, FO, P, H).transpose(0, 2, 1, 3)
        )  # [S, 128, FO, H]
        # xT[s, hi, ko, t] = x_slot_s[t, ko*128+hi], zero-padded to Cmax
        Cmax = int(max(C))
        xT_c = np.zeros((SLOTS, P, KO, Cmax), bf16)
        for s, e in enumerate(experts):
            n = int(sizes[e])
            xs = x[tok_offs[e] : tok_offs[e] + n]  # [n, H]
            xT_c[s, :, :, :n] = xs.T.reshape(KO, P, n).transpose(1, 0, 2)
        in_maps.append({"w1s": w1s, "w2s": w2s, "xT": xT_c})
    return in_maps


def scatter_output(results, sizes, expert_of, offs):
    """Gather per-core [128, OO, CTOT] bf16 outputs into full [T, H] f32."""
    tok_offs = np.concatenate([[0], np.cumsum(sizes)]).astype(np.int64)
    out = np.empty((T, H), np.float32)
    for c in range(NCORES):
        outT_c = np.asarray(results[c]["outT"])  # [128, OO, CTOT] bf16
        for s, e in enumerate(expert_of[c]):
            n = int(sizes[e])
            blk = outT_c[:, :, offs[s] : offs[s] + n]  # [oi, oo, t]
            out[tok_offs[e] : tok_offs[e] + n] = (
                blk.transpose(2, 1, 0).reshape(n, H).astype(np.float32)
            )
    return out


def kernel(x, w1, w2, tokens_per_expert):
    from concourse import bass2jax

    sizes = np.asarray(tokens_per_expert, np.int64)
    expert_of, C, offs = plan(sizes)
    nc = _get_program(C)
    in_maps = prepare_inputs(x, w1, w2, sizes, expert_of, C, offs)
    results = bass2jax.run_bass_via_pjrt(nc, in_maps, n_cores=NCORES)
    return scatter_output(results, sizes, expert_of, offs)
